# revision 5
# baseline (speedup 1.0000x reference)
"""Distributed Bass kernel for masked multi-head self-attention on 8 TRN2 NeuronCores.

Problem: x[2,2048,1024] -> qkv -> 16-head attention with outer-product mask
(keep[i,j] = mask[i]*mask[j]) -> out proj.  Key observation: masked queries
produce exactly 0 rows and masked keys are fully excluded, so only the
~m unmasked tokens per batch participate.  Host-side we compact tokens per
batch, pad to a multiple of 128, and run sequence-parallel attention:
cores 0-3 own batch 0, cores 4-7 batch 1; each core computes QKV for its
mq-token slice, AllGathers K^T/V within its 4-core group, then computes
full attention + out-proj for its queries.

Padded token slots have x=0, so their keys score exp(0)=1 against every
query; the softmax denominator is corrected by subtracting the pad count
(runtime input), which keeps the device graph identical across cores.
Compute dtype is bf16 (f32 PSUM accumulation); softmax runs without
max-subtraction (scores are O(5), exp is safe in f32).
"""

import math
from contextlib import ExitStack

import numpy as np
import ml_dtypes

import concourse.bass as bass
import concourse.mybir as mybir
import concourse.tile as tile
from concourse import bacc
from concourse.bass_utils import run_bass_kernel_spmd
from concourse.masks import make_identity

P = 128
HEADS = 16
DH = 64
D = 1024          # model dim
INNER = 1024      # heads * dh
SCALE = DH ** -0.5
N_CORES = 8
RPB = 4           # ranks (cores) per batch
GROUPS = [[0, 1, 2, 3], [4, 5, 6, 7]]
BF16 = mybir.dt.bfloat16
F32 = mybir.dt.float32


def _build(mq: int, dbg: bool = False):
    """Build the per-core SPMD graph for mq queries/core (mq % 32 == 0)."""
    Kk = RPB * mq               # key slots per batch, multiple of 128
    nkt = Kk // P               # 128-row key tiles
    TT = math.ceil(mq / P)      # query-token tiles per core

    def tspan(tt):
        return min(P, mq - tt * P)

    nc = bacc.Bacc(None, target_bir_lowering=False, num_devices=N_CORES)

    dbg_outs = {}

    def mkdbg(name, shape):
        if dbg:
            dbg_outs[name] = nc.declare_dram_parameter(name, list(shape), F32,
                                                       isOutput=True)

    mkdbg("dbg_xt", [P, mq])
    mkdbg("dbg_k", [P, mq])
    mkdbg("dbg_kf", [P, RPB * mq])
    mkdbg("dbg_vt", [P, HEADS * (DH + 1)])
    mkdbg("dbg_q", [P, mq])
    mkdbg("dbg_pt", [P, 3 * mq])
    mkdbg("dbg_av", [DH + 1, mq])
    mkdbg("dbg_ao", [P, mq])

    x_in = nc.declare_dram_parameter("x", [mq, D], BF16, isOutput=False)
    wqkv_in = nc.declare_dram_parameter("wqkv", [D, 3 * INNER], BF16, isOutput=False)
    wout_in = nc.declare_dram_parameter("wout", [INNER, D], BF16, isOutput=False)
    npad_in = nc.declare_dram_parameter("npad", [1, 1], F32, isOutput=False)
    out_ext = nc.declare_dram_parameter("out", [mq, D], F32, isOutput=True)

    BLK = INNER * mq            # elements of one K^T or V block in the bounce

    with tile.TileContext(nc) as tc, ExitStack() as ctx:
        sb = ctx.enter_context(tc.tile_pool(name="sb", bufs=1))
        ps = ctx.enter_context(tc.tile_pool(name="ps", bufs=1, space="PSUM"))
        dramp = ctx.enter_context(tc.tile_pool(name="dram", bufs=1, space="DRAM"))

        inb = dramp.tile([2 * BLK], BF16, name="inb")
        outb = dramp.tile([RPB * 2 * BLK], BF16, name="outb")
        kb_view = inb[0:BLK].rearrange("(a b) -> a b", b=mq)              # [1024, mq]
        vb_view = inb[BLK:2 * BLK].rearrange("(a b) -> a b", b=D)         # [mq, 1024]

        def probe(name, src_ap):
            if not dbg:
                return
            p_, w_ = src_ap.shape
            t_ = sb.tile([p_, w_], F32, tag=f"probe_{name}", bufs=1, name=f"pb_{name}")
            nc.vector.tensor_copy(t_[:], src_ap)
            nc.sync.dma_start(dbg_outs[name][0:p_, 0:w_], t_[:])

        ident = sb.tile([P, P], BF16, tag="ident", bufs=1, name="ident")
        make_identity(nc, ident[:])

        npad_sb = sb.tile([1, 1], F32, tag="npad", bufs=1, name="npad_sb")
        nc.sync.dma_start(npad_sb[:], npad_in[:])

        # ---- Phase A: load x rows, transpose to xT (model-dim on partitions)
        xr = []
        for tt in range(TT):
            t_ = sb.tile([P, D], BF16, tag="xr", bufs=TT, name=f"xr{tt}")
            pt = tspan(tt)
            nc.sync.dma_start(t_[0:pt, :], x_in[tt * P: tt * P + pt, :])
            xr.append(t_)
        xT = []
        for ft in range(8):
            t_ = sb.tile([P, mq], BF16, tag="xT", bufs=8, name=f"xT{ft}")
            xT.append(t_)
        for ft in range(8):
            for tt in range(TT):
                pt = tspan(tt)
                tp = ps.tile([P, P], BF16, tag="mm", name=f"tp{ft}_{tt}")
                nc.tensor.transpose(tp[:], xr[tt][:, ft * P:(ft + 1) * P], ident[:])
                nc.vector.tensor_copy(xT[ft][:, tt * P: tt * P + pt], tp[:, 0:pt])
        probe("dbg_xt", xT[0][:])

        # ---- Phase B: QKV.  K and V first (feeds AllGather), Q overlaps the AG.
        wk, wv, wq = [], [], []
        for kc in range(8):
            tk = sb.tile([P, INNER], BF16, tag="wk", bufs=8, name=f"wk{kc}")
            nc.sync.dma_start(tk[:], wqkv_in[kc * P:(kc + 1) * P, INNER:2 * INNER])
            wk.append(tk)
        for kc in range(8):
            tv = sb.tile([P, INNER], BF16, tag="wv", bufs=8, name=f"wv{kc}")
            nc.sync.dma_start(tv[:], wqkv_in[kc * P:(kc + 1) * P, 2 * INNER:3 * INNER])
            wv.append(tv)
        for kc in range(8):
            tq = sb.tile([P, INNER], BF16, tag="wq", bufs=8, name=f"wq{kc}")
            nc.sync.dma_start(tq[:], wqkv_in[kc * P:(kc + 1) * P, 0:INNER])
            wq.append(tq)

        # K^T  [feat_tile 128, mq] -> bounce
        for t in range(8):
            kps = ps.tile([P, mq], F32, tag="mm", name=f"kps{t}")
            for kc in range(8):
                nc.tensor.matmul(kps[:], wk[kc][:, t * P:(t + 1) * P], xT[kc][:],
                                 start=(kc == 0), stop=(kc == 7))
            kloc = sb.tile([P, mq], BF16, tag="kloc", bufs=4, name=f"kloc{t}")
            nc.vector.tensor_copy(kloc[:], kps[:])
            nc.sync.dma_start(kb_view[t * P:(t + 1) * P, :], kloc[:])
            if t == 0:
                probe("dbg_k", kloc[:])

        # V    [tok_tile, 1024] -> bounce
        for tt in range(TT):
            pt = tspan(tt)
            vloc = sb.tile([P, D], BF16, tag="vloc", bufs=TT, name=f"vloc{tt}")
            for nf in range(2):
                vps = ps.tile([P, 512], F32, tag="mm", name=f"vps{tt}_{nf}")
                for kc in range(8):
                    nc.tensor.matmul(vps[0:pt, :], xT[kc][:, tt * P: tt * P + pt],
                                     wv[kc][:, nf * 512:(nf + 1) * 512],
                                     start=(kc == 0), stop=(kc == 7))
                nc.vector.tensor_copy(vloc[0:pt, nf * 512:(nf + 1) * 512], vps[0:pt, :])
            nc.sync.dma_start(vb_view[tt * P: tt * P + pt, :], vloc[0:pt, :])

        # V~ tiles: [128 keys, 16*(64+1)] with a ones column per head (softmax
        # denominator rides along row 64 of each head's AV matmul).  Memsets are
        # independent of the AG, so emit them before it.
        vt = []
        for kt in range(nkt):
            t_ = sb.tile([P, HEADS * (DH + 1)], BF16, tag="vt", bufs=nkt, name=f"vt{kt}")
            nc.gpsimd.memset(t_[:, :], 0.0)
            nc.gpsimd.memset(
                t_[:].rearrange("p (h c) -> p h c", c=DH + 1)[:, :, DH:DH + 1], 1.0)
            vt.append(t_)

        nc.gpsimd.collective_compute(
            "AllGather", mybir.AluOpType.bypass, replica_groups=GROUPS,
            ins=[inb.opt()], outs=[outb.opt()],
        )

        # Q^T, zero-padded per head: qtz[h] has head h's 64 dims in the same
        # partition rows they occupy in the packed feature tile, zeros in the
        # other 64 rows.  S^T matmuls then contract over the full 128 rows and
        # share one lhsT (the packed K^T tile) between both heads of a pair.
        qtz = [None] * HEADS
        for t in range(8):
            qps = ps.tile([P, mq], F32, tag="mm", name=f"qps{t}")
            for kc in range(8):
                nc.tensor.matmul(qps[:], wq[kc][:, t * P:(t + 1) * P], xT[kc][:],
                                 start=(kc == 0), stop=(kc == 7))
            a = sb.tile([P, mq], BF16, tag="qtz", bufs=HEADS, name=f"qtz{2 * t}")
            nc.vector.memset(a[64:128, :], 0.0)
            nc.vector.tensor_copy(a[0:64, :], qps[0:64, :])
            qtz[2 * t] = a
            b = sb.tile([P, mq], BF16, tag="qtz", bufs=HEADS, name=f"qtz{2 * t + 1}")
            nc.vector.memset(b[0:64, :], 0.0)
            nc.vector.tensor_copy(b[64:128, :], qps[64:128, :])
            qtz[2 * t + 1] = b
        probe("dbg_q", qtz[0][:])

        # ---- Phase C: read gathered K^T / V from the bounce.
        kf = [None] * 8

        def load_kf(t):
            t_ = sb.tile([P, Kk], BF16, tag="kf", bufs=8, name=f"kf{t}")
            for r in range(RPB):
                off = r * 2 * BLK
                view = outb[off: off + BLK].rearrange("(a b) -> a b", b=mq)
                nc.sync.dma_start(t_[:, r * mq:(r + 1) * mq],
                                  view[t * P:(t + 1) * P, :])
            kf[t] = t_

        load_kf(0)
        # V rows kt*128..+128 may straddle rank boundaries; split DMAs per rank.
        for kt in range(nkt):
            g0 = kt * P
            end = g0 + P
            while g0 < end:
                r = g0 // mq
                g1 = min((r + 1) * mq, end)
                off = r * 2 * BLK + BLK + (g0 - r * mq) * D
                src = outb[off: off + (g1 - g0) * D].rearrange(
                    "(a b) -> a b", b=D).rearrange("p (h c) -> p h c", c=DH)
                dst = vt[kt][g0 - kt * P: g1 - kt * P, :].rearrange(
                    "p (h c) -> p h c", c=DH + 1)[:, :, 0:DH]
                nc.sync.dma_start(dst, src)
                g0 = g1
        for t in range(1, 8):
            load_kf(t)
        probe("dbg_kf", kf[0][:])
        probe("dbg_vt", vt[0][:])

        # ---- Phase D: attention, one head-pair at a time.
        # units = (key-tile, head01); 3 units per PSUM staging tile (3 banks).
        CH = 3
        aoT = []
        for hp in range(8):
            ha, hb = 2 * hp, 2 * hp + 1
            avp_a = ps.tile([DH + 1, mq], F32, tag="av", bufs=2, name=f"av{hp}a")
            avp_b = ps.tile([DH + 1, mq], F32, tag="av", bufs=2, name=f"av{hp}b")
            avp = [avp_a, avp_b]
            units = [(kt, h) for kt in range(nkt) for h in (0, 1)]
            for g0 in range(0, len(units), CH):
                grp = units[g0:g0 + CH]
                sps = ps.tile([P, CH * 512], F32, tag="mm", name=f"sps{hp}_{g0}")
                for j, (kt, h) in enumerate(grp):
                    nc.tensor.matmul(sps[:, j * 512: j * 512 + mq],
                                     kf[hp][:, kt * P:(kt + 1) * P],
                                     qtz[2 * hp + h][:],
                                     start=True, stop=True)
                pt_ = sb.tile([P, CH * mq], BF16, tag="pt", bufs=3, name=f"pt{hp}_{g0}")
                nc.scalar.activation(
                    pt_[:].rearrange("p (u c) -> p u c", c=mq)[:, 0:len(grp), :],
                    sps[:].rearrange("p (u c) -> p u c", c=512)[:, 0:len(grp), 0:mq],
                    mybir.ActivationFunctionType.Exp, scale=SCALE)
                if hp == 0 and g0 == 0:
                    probe("dbg_pt", pt_[:])
                for j, (kt, h) in enumerate(grp):
                    nc.tensor.matmul(
                        avp[h][:],
                        vt[kt][:, (2 * hp + h) * (DH + 1):(2 * hp + h + 1) * (DH + 1)],
                        pt_[:, j * mq:(j + 1) * mq],
                        start=(kt == 0), stop=(kt == nkt - 1),
                        skip_group_check=True)

            if hp == 0:
                probe("dbg_av", avp_a[:])
            # softmax denominators ride in row 64; subtract the pad count,
            # reciprocal, broadcast across the 64 head dims, scale, pack.
            ao = sb.tile([P, mq], BF16, tag="aoT", bufs=8, name=f"aoT{hp}")
            for h, (av_, hglob) in enumerate(((avp_a, ha), (avp_b, hb))):
                den = sb.tile([1, mq], F32, tag="den", bufs=4, name=f"den{hp}_{h}")
                nc.vector.tensor_scalar(den[:], av_[DH:DH + 1, :], npad_sb[0:1, 0:1],
                                        None, op0=mybir.AluOpType.subtract)
                rec = sb.tile([1, mq], F32, tag="rec", bufs=4, name=f"rec{hp}_{h}")
                nc.vector.reciprocal(rec[:], den[:])
                fac = sb.tile([DH, mq], F32, tag="fac", bufs=4, name=f"fac{hp}_{h}")
                nc.gpsimd.partition_broadcast(fac[:], rec[:])
                if h == 0:
                    nc.vector.tensor_tensor(ao[0:DH, :], av_[0:DH, :], fac[:],
                                            op=mybir.AluOpType.mult)
                else:
                    tmpb = sb.tile([DH, mq], BF16, tag="tmpb", bufs=2,
                                   name=f"tmpb{hp}")
                    nc.vector.tensor_tensor(tmpb[:], av_[0:DH, :], fac[:],
                                            op=mybir.AluOpType.mult)
                    # partition shift 0:64 -> 64:128 needs a DMA, not DVE
                    nc.sync.dma_start(ao[DH:P, :], tmpb[:])
            if hp == 0:
                probe("dbg_ao", ao[:])
            aoT.append(ao)

        # ---- Phase E: out projection.
        wout_sb = []
        for t in range(8):
            tw = sb.tile([P, D], BF16, tag="wout", bufs=8, name=f"wo{t}")
            nc.sync.dma_start(tw[:], wout_in[t * P:(t + 1) * P, :])
            wout_sb.append(tw)
        for mt in range(TT):
            pm = tspan(mt)
            osb = sb.tile([P, D], F32, tag="osb", bufs=2, name=f"osb{mt}")
            for nf in range(2):
                op_ps = ps.tile([P, 512], F32, tag="mm", name=f"op{mt}_{nf}")
                for t in range(8):
                    nc.tensor.matmul(op_ps[0:pm, :], aoT[t][:, mt * P: mt * P + pm],
                                     wout_sb[t][:, nf * 512:(nf + 1) * 512],
                                     start=(t == 0), stop=(t == 7))
                nc.vector.tensor_copy(osb[0:pm, nf * 512:(nf + 1) * 512],
                                      op_ps[0:pm, :])
            nc.sync.dma_start(out_ext[mt * P: mt * P + pm, :], osb[0:pm, :])

    nc.compile()
    return nc


_GRAPH_CACHE: dict = {}


def _get_graph(mq: int):
    if mq not in _GRAPH_CACHE:
        _GRAPH_CACHE[mq] = _build(mq)
    return _GRAPH_CACHE[mq]


def kernel(x, mask, W_qkv, W_out):
    x = np.asarray(x, dtype=np.float32)
    mask = np.asarray(mask, dtype=np.float32)
    W_qkv = np.asarray(W_qkv, dtype=np.float32)
    W_out = np.asarray(W_out, dtype=np.float32)
    b, n, d = x.shape
    assert (b, d) == (2, D) and W_qkv.shape == (D, 3 * INNER)

    idx = [np.nonzero(mask[i] > 0.5)[0] for i in range(b)]
    m = [len(ix) for ix in idx]
    mq = max(32, math.ceil(max(m) / RPB / 32) * 32)
    Kk = RPB * mq

    nc = _get_graph(mq)

    bf16 = ml_dtypes.bfloat16
    xg = np.zeros((b, Kk, d), dtype=np.float32)
    for i in range(b):
        xg[i, :m[i]] = x[i][idx[i]]
    xg = xg.astype(bf16)
    wqkv_bf = W_qkv.astype(bf16)
    wout_bf = W_out.astype(bf16)

    in_maps = []
    for core in range(N_CORES):
        bi, r = divmod(core, RPB)
        in_maps.append({
            "x": np.ascontiguousarray(xg[bi, r * mq:(r + 1) * mq]),
            "wqkv": wqkv_bf,
            "wout": wout_bf,
            "npad": np.array([[Kk - m[bi]]], dtype=np.float32),
        })

    res = run_bass_kernel_spmd(nc, in_maps, core_ids=list(range(N_CORES)))

    out = np.zeros((b, n, d), dtype=np.float32)
    for bi in range(b):
        cat = np.concatenate(
            [res.results[bi * RPB + r]["out"] for r in range(RPB)], axis=0)
        out[bi][idx[bi]] = cat[:m[bi]]
    return out


# revision 6
# speedup vs baseline: 1.0124x; 1.0124x over previous
"""Distributed Bass kernel for masked multi-head self-attention on 8 TRN2 NeuronCores.

Problem: x[2,2048,1024] -> qkv -> 16-head attention with outer-product mask
(keep[i,j] = mask[i]*mask[j]) -> out proj.  Masked queries produce exactly 0
rows and masked keys are fully excluded, so only the ~m unmasked tokens per
batch participate.  Host-side we compact tokens per batch, pad to a multiple
of 128 key slots (Kk), and split the batch dimension across the two 4-core
groups; within a group each core owns mq = Kk/4 query tokens.

Collectives on this part have a ~60-120us floor, far more than the ~25us of
extra TensorE time it takes to just recompute K and V for the whole batch on
every core of the group - so each core computes full-batch K^T and V locally
(x^T arrives host-pretransposed; no collective, no bounce buffers).

Padded token slots have x=0, so their keys score exp(0)=1 against every
query; the softmax denominator is corrected by subtracting the pad count
(a runtime input, keeping the device graph identical across cores).
Compute dtype is bf16 (f32 PSUM accumulation); softmax runs without
max-subtraction (scores are O(5), exp is safe in f32).
"""

import math
from contextlib import ExitStack

import numpy as np
import ml_dtypes

import concourse.bass as bass
import concourse.mybir as mybir
import concourse.tile as tile
from concourse import bacc
from concourse.bass_utils import run_bass_kernel_spmd

P = 128
HEADS = 16
DH = 64
D = 1024          # model dim
INNER = 1024      # heads * dh
SCALE = DH ** -0.5
N_CORES = 8
RPB = 4           # ranks (cores) per batch
BF16 = mybir.dt.bfloat16
F32 = mybir.dt.float32


def _build(mq: int, dbg: bool = False):
    """Build the per-core SPMD graph for mq queries/core (mq % 32 == 0)."""
    Kk = RPB * mq               # key slots per batch, multiple of 128
    nkt = Kk // P               # 128-row key tiles
    TT = math.ceil(mq / P)      # query-token tiles per core
    KCH = 384                   # K^T free-dim chunk (psum-bank friendly)
    nkch = math.ceil(Kk / KCH)

    def tspan(tt):
        return min(P, mq - tt * P)

    nc = bacc.Bacc(None, target_bir_lowering=False, num_devices=N_CORES)

    xt_in = nc.declare_dram_parameter("xt", [D, Kk], BF16, isOutput=False)
    xqt_in = nc.declare_dram_parameter("xqt", [D, mq], BF16, isOutput=False)
    wqkv_in = nc.declare_dram_parameter("wqkv", [D, 3 * INNER], BF16, isOutput=False)
    wout_in = nc.declare_dram_parameter("wout", [INNER, D], BF16, isOutput=False)
    npad_in = nc.declare_dram_parameter("npad", [1, 1], F32, isOutput=False)
    out_ext = nc.declare_dram_parameter("out", [mq, D], F32, isOutput=True)

    with tile.TileContext(nc) as tc, ExitStack() as ctx:
        sb = ctx.enter_context(tc.tile_pool(name="sb", bufs=1))
        ps = ctx.enter_context(tc.tile_pool(name="ps", bufs=1, space="PSUM"))

        npad_sb = sb.tile([1, 1], F32, tag="npad", bufs=1, name="npad_sb")
        nc.sync.dma_start(npad_sb[:], npad_in[:])

        # ---- inputs: x^T (full batch + own query slice), weights
        xt, xqt = [], []
        for kc in range(8):
            t_ = sb.tile([P, Kk], BF16, tag="xt", bufs=8, name=f"xt{kc}")
            nc.sync.dma_start(t_[:, 0:Kk // 2], xt_in[kc * P:(kc + 1) * P, 0:Kk // 2])
            nc.sync.dma_start(t_[:, Kk // 2:Kk],
                              xt_in[kc * P:(kc + 1) * P, Kk // 2:Kk])
            xt.append(t_)
            tq = sb.tile([P, mq], BF16, tag="xqt", bufs=8, name=f"xqt{kc}")
            nc.sync.dma_start(tq[:], xqt_in[kc * P:(kc + 1) * P, :])
            xqt.append(tq)

        wk, wv, wq = [], [], []
        for kc in range(8):
            tk = sb.tile([P, INNER], BF16, tag="wk", bufs=8, name=f"wk{kc}")
            nc.sync.dma_start(tk[:, 0:512], wqkv_in[kc * P:(kc + 1) * P, INNER:INNER + 512])
            nc.sync.dma_start(tk[:, 512:1024],
                              wqkv_in[kc * P:(kc + 1) * P, INNER + 512:2 * INNER])
            wk.append(tk)
        for kc in range(8):
            tv = sb.tile([P, INNER], BF16, tag="wv", bufs=8, name=f"wv{kc}")
            nc.sync.dma_start(tv[:, 0:512],
                              wqkv_in[kc * P:(kc + 1) * P, 2 * INNER:2 * INNER + 512])
            nc.sync.dma_start(tv[:, 512:1024],
                              wqkv_in[kc * P:(kc + 1) * P, 2 * INNER + 512:3 * INNER])
            wv.append(tv)
        for kc in range(8):
            tq_ = sb.tile([P, INNER], BF16, tag="wq", bufs=8, name=f"wq{kc}")
            nc.sync.dma_start(tq_[:, 0:512], wqkv_in[kc * P:(kc + 1) * P, 0:512])
            nc.sync.dma_start(tq_[:, 512:1024], wqkv_in[kc * P:(kc + 1) * P, 512:INNER])
            wq.append(tq_)

        # ---- K^T for the whole batch: kf[t] [128 featdims, Kk keys] bf16
        kf = []
        for t in range(8):
            kft = sb.tile([P, Kk], BF16, tag="kf", bufs=8, name=f"kf{t}")
            for ch in range(nkch):
                w_ = min(KCH, Kk - ch * KCH)
                kps = ps.tile([P, KCH], F32, tag="mm", name=f"kps{t}_{ch}")
                for kc in range(8):
                    nc.tensor.matmul(kps[:, 0:w_], wk[kc][:, t * P:(t + 1) * P],
                                     xt[kc][:, ch * KCH: ch * KCH + w_],
                                     start=(kc == 0), stop=(kc == 7))
                nc.vector.tensor_copy(kft[:, ch * KCH: ch * KCH + w_], kps[:, 0:w_])
            kf.append(kft)

        # ---- V~ for the whole batch: vt[kt] [128 keys, 16*(64+1)] bf16 with a
        # ones column per head (softmax denominator rides row 64 of AV psum).
        vt = []
        for kt in range(nkt):
            t_ = sb.tile([P, HEADS * (DH + 1)], BF16, tag="vt", bufs=nkt, name=f"vt{kt}")
            nc.gpsimd.memset(
                t_[:].rearrange("p (h c) -> p h c", c=DH + 1)[:, :, DH:DH + 1], 1.0)
            for nf in range(2):
                vps = ps.tile([P, 512], F32, tag="mm", name=f"vps{kt}_{nf}")
                for kc in range(8):
                    nc.tensor.matmul(vps[:], xt[kc][:, kt * P:(kt + 1) * P],
                                     wv[kc][:, nf * 512:(nf + 1) * 512],
                                     start=(kc == 0), stop=(kc == 7))
                nc.vector.tensor_copy(
                    t_[:].rearrange("p (h c) -> p h c", c=DH + 1)[:, nf * 8:(nf + 1) * 8, 0:DH],
                    vps[:].rearrange("p (h c) -> p h c", c=DH))
            vt.append(t_)

        # ---- Q^T (own slice), zero-padded per head: qtz[h] has head h's 64
        # dims in their packed partition rows, zeros in the other 64, so S^T
        # contracts over the full 128 rows sharing one K^T lhsT per head pair.
        qtz = [None] * HEADS
        for t in range(8):
            qps = ps.tile([P, mq], F32, tag="mm", name=f"qps{t}")
            for kc in range(8):
                nc.tensor.matmul(qps[:], wq[kc][:, t * P:(t + 1) * P], xqt[kc][:],
                                 start=(kc == 0), stop=(kc == 7))
            a = sb.tile([P, mq], BF16, tag="qtz", bufs=HEADS, name=f"qtz{2 * t}")
            nc.vector.memset(a[64:128, :], 0.0)
            nc.vector.tensor_copy(a[0:64, :], qps[0:64, :])
            qtz[2 * t] = a
            b = sb.tile([P, mq], BF16, tag="qtz", bufs=HEADS, name=f"qtz{2 * t + 1}")
            nc.vector.memset(b[0:64, :], 0.0)
            nc.vector.tensor_copy(b[64:128, :], qps[64:128, :])
            qtz[2 * t + 1] = b

        # ---- attention, one head-pair at a time.
        # units = (key-tile, head01); 3 units per PSUM staging tile (3 banks).
        CH = 3
        aoT = []
        for hp in range(8):
            avp_a = ps.tile([DH + 1, mq], F32, tag="av", bufs=2, name=f"av{hp}a")
            avp_b = ps.tile([DH + 1, mq], F32, tag="av", bufs=2, name=f"av{hp}b")
            avp = [avp_a, avp_b]
            units = [(kt, h) for kt in range(nkt) for h in (0, 1)]
            for g0 in range(0, len(units), CH):
                grp = units[g0:g0 + CH]
                sps = ps.tile([P, CH * 512], F32, tag="mm", name=f"sps{hp}_{g0}")
                for j, (kt, h) in enumerate(grp):
                    nc.tensor.matmul(sps[:, j * 512: j * 512 + mq],
                                     kf[hp][:, kt * P:(kt + 1) * P],
                                     qtz[2 * hp + h][:],
                                     start=True, stop=True)
                pt_ = sb.tile([P, CH * mq], BF16, tag="pt", bufs=3, name=f"pt{hp}_{g0}")
                nc.scalar.activation(
                    pt_[:].rearrange("p (u c) -> p u c", c=mq)[:, 0:len(grp), :],
                    sps[:].rearrange("p (u c) -> p u c", c=512)[:, 0:len(grp), 0:mq],
                    mybir.ActivationFunctionType.Exp, scale=SCALE)
                for j, (kt, h) in enumerate(grp):
                    nc.tensor.matmul(
                        avp[h][:],
                        vt[kt][:, (2 * hp + h) * (DH + 1):(2 * hp + h + 1) * (DH + 1)],
                        pt_[:, j * mq:(j + 1) * mq],
                        start=(kt == 0), stop=(kt == nkt - 1),
                        skip_group_check=True)

            # softmax denominators ride in row 64; subtract the pad count,
            # reciprocal, broadcast across the 64 head dims, scale, pack.
            ao = sb.tile([P, mq], BF16, tag="aoT", bufs=8, name=f"aoT{hp}")
            for h, av_ in enumerate((avp_a, avp_b)):
                den = sb.tile([1, mq], F32, tag="den", bufs=4, name=f"den{hp}_{h}")
                nc.vector.tensor_scalar(den[:], av_[DH:DH + 1, :], npad_sb[0:1, 0:1],
                                        None, op0=mybir.AluOpType.subtract)
                rec = sb.tile([1, mq], F32, tag="rec", bufs=4, name=f"rec{hp}_{h}")
                nc.vector.reciprocal_approx_fast(rec[:], den[:])
                fac = sb.tile([DH, mq], F32, tag="fac", bufs=4, name=f"fac{hp}_{h}")
                nc.gpsimd.partition_broadcast(fac[:], rec[:])
                if h == 0:
                    nc.vector.tensor_tensor(ao[0:DH, :], av_[0:DH, :], fac[:],
                                            op=mybir.AluOpType.mult)
                else:
                    tmpb = sb.tile([DH, mq], BF16, tag="tmpb", bufs=2,
                                   name=f"tmpb{hp}")
                    nc.vector.tensor_tensor(tmpb[:], av_[0:DH, :], fac[:],
                                            op=mybir.AluOpType.mult)
                    # partition shift 0:64 -> 64:128 needs a DMA, not DVE
                    nc.sync.dma_start(ao[DH:P, :], tmpb[:])
            aoT.append(ao)

        # ---- out projection.
        wout_sb = []
        for t in range(8):
            tw = sb.tile([P, D], BF16, tag="wout", bufs=8, name=f"wo{t}")
            nc.sync.dma_start(tw[:, 0:512], wout_in[t * P:(t + 1) * P, 0:512])
            nc.sync.dma_start(tw[:, 512:1024], wout_in[t * P:(t + 1) * P, 512:1024])
            wout_sb.append(tw)
        for mt in range(TT):
            pm = tspan(mt)
            osb = sb.tile([P, D], F32, tag="osb", bufs=2, name=f"osb{mt}")
            for nf in range(2):
                op_ps = ps.tile([P, 512], F32, tag="mm", name=f"op{mt}_{nf}")
                for t in range(8):
                    nc.tensor.matmul(op_ps[0:pm, :], aoT[t][:, mt * P: mt * P + pm],
                                     wout_sb[t][:, nf * 512:(nf + 1) * 512],
                                     start=(t == 0), stop=(t == 7))
                nc.vector.tensor_copy(osb[0:pm, nf * 512:(nf + 1) * 512],
                                      op_ps[0:pm, :])
            nc.sync.dma_start(out_ext[mt * P: mt * P + pm, :], osb[0:pm, :])

    nc.compile()
    return nc


_GRAPH_CACHE: dict = {}


def _get_graph(mq: int):
    if mq not in _GRAPH_CACHE:
        _GRAPH_CACHE[mq] = _build(mq)
    return _GRAPH_CACHE[mq]


def kernel(x, mask, W_qkv, W_out):
    x = np.asarray(x, dtype=np.float32)
    mask = np.asarray(mask, dtype=np.float32)
    W_qkv = np.asarray(W_qkv, dtype=np.float32)
    W_out = np.asarray(W_out, dtype=np.float32)
    b, n, d = x.shape
    assert (b, d) == (2, D) and W_qkv.shape == (D, 3 * INNER)

    idx = [np.nonzero(mask[i] > 0.5)[0] for i in range(b)]
    m = [len(ix) for ix in idx]
    mq = max(32, math.ceil(max(m) / RPB / 32) * 32)
    Kk = RPB * mq

    nc = _get_graph(mq)

    bf16 = ml_dtypes.bfloat16
    xg = np.zeros((b, Kk, d), dtype=np.float32)
    for i in range(b):
        xg[i, :m[i]] = x[i][idx[i]]
    xgT = np.ascontiguousarray(xg.astype(bf16).transpose(0, 2, 1))  # [b, D, Kk]
    wqkv_bf = W_qkv.astype(bf16)
    wout_bf = W_out.astype(bf16)

    in_maps = []
    for core in range(N_CORES):
        bi, r = divmod(core, RPB)
        in_maps.append({
            "xt": xgT[bi],
            "xqt": np.ascontiguousarray(xgT[bi][:, r * mq:(r + 1) * mq]),
            "wqkv": wqkv_bf,
            "wout": wout_bf,
            "npad": np.array([[Kk - m[bi]]], dtype=np.float32),
        })

    res = run_bass_kernel_spmd(nc, in_maps, core_ids=list(range(N_CORES)))

    out = np.zeros((b, n, d), dtype=np.float32)
    for bi in range(b):
        cat = np.concatenate(
            [res.results[bi * RPB + r]["out"] for r in range(RPB)], axis=0)
        out[bi][idx[bi]] = cat[:m[bi]]
    return out


# revision 11
# speedup vs baseline: 1.1707x; 1.1563x over previous
"""Distributed Bass kernel for masked multi-head self-attention on 8 TRN2 NeuronCores.

Problem: x[2,2048,1024] -> qkv -> 16-head attention with outer-product mask
(keep[i,j] = mask[i]*mask[j]) -> out proj.  Masked queries produce exactly 0
rows and masked keys are fully excluded, so only the ~m unmasked tokens per
batch participate.  Host-side we compact tokens per batch, pad to a multiple
of 128 key slots (Kk), and split the batch dimension across the two 4-core
groups; within a group each core owns mq = Kk/4 query tokens.

Collectives on this part have a ~60-120us floor, far more than the ~25us of
extra TensorE time it takes to just recompute K and V for the whole batch on
every core of the group - so each core computes full-batch K^T and V locally
(x^T arrives host-pretransposed; no collective, no bounce buffers).

Padded token slots have x=0, so their keys score exp(0)=1 against every
query; the softmax denominator is corrected by subtracting the pad count
(a runtime input, keeping the device graph identical across cores).
Compute dtype is bf16 (f32 PSUM accumulation); softmax runs without
max-subtraction (scores are O(5), exp is safe in f32).
"""

import math
from contextlib import ExitStack

import numpy as np
import ml_dtypes

import concourse.bass as bass
import concourse.mybir as mybir
import concourse.tile as tile
from concourse import bacc
from concourse.bass_utils import run_bass_kernel_spmd
P = 128
HEADS = 16
DH = 64
D = 1024          # model dim
INNER = 1024      # heads * dh
SCALE = DH ** -0.5
N_CORES = 8
RPB = 4           # ranks (cores) per batch
BF16 = mybir.dt.bfloat16
F32 = mybir.dt.float32


def _build(mq: int, dbg: bool = False):
    """Build the per-core SPMD graph for mq queries/core (mq % 32 == 0)."""
    Kk = RPB * mq               # key slots per batch, multiple of 128
    nkt = Kk // P               # 128-row key tiles
    TT = math.ceil(mq / P)      # query-token tiles per core
    KCH = 384                   # K^T free-dim chunk (psum-bank friendly)
    nkch = math.ceil(Kk / KCH)

    def tspan(tt):
        return min(P, mq - tt * P)

    nc = bacc.Bacc(None, target_bir_lowering=False, num_devices=N_CORES)

    xt_in = nc.declare_dram_parameter("xt", [D, Kk], BF16, isOutput=False)
    xqt_in = nc.declare_dram_parameter("xqt", [D, mq], BF16, isOutput=False)
    wqkv_in = nc.declare_dram_parameter("wqkv", [D, 3 * INNER], BF16, isOutput=False)
    wout_in = nc.declare_dram_parameter("wout", [INNER, D], BF16, isOutput=False)
    npad_in = nc.declare_dram_parameter("npad", [1, 1], F32, isOutput=False)
    out_ext = nc.declare_dram_parameter("out", [mq, D], F32, isOutput=True)

    with tile.TileContext(nc) as tc, ExitStack() as ctx:
        sb = ctx.enter_context(tc.tile_pool(name="sb", bufs=1))
        ps = ctx.enter_context(tc.tile_pool(name="ps", bufs=1, space="PSUM"))

        npad_sb = sb.tile([1, 1], F32, tag="npad", bufs=1, name="npad_sb")
        nc.sync.dma_start(npad_sb[:], npad_in[:])

        # ---- inputs: x^T (full batch + own query slice), weights.
        # Round-robin the DMA issues over four sequencers (issue costs ~600ns
        # serially per sequencer); wk+xt first, they gate the first matmul.
        seqs = [nc.sync, nc.scalar, nc.gpsimd]
        _n = [0]

        def dma(dst, src):
            seqs[_n[0] % len(seqs)].dma_start(dst, src)
            _n[0] += 1

        xt, xqt, wk = [], [], []
        for kc in range(8):
            tk = sb.tile([P, INNER], BF16, tag="wk", bufs=8, name=f"wk{kc}")
            dma(tk[:], wqkv_in[kc * P:(kc + 1) * P, INNER:2 * INNER])
            wk.append(tk)
            t_ = sb.tile([P, Kk], BF16, tag="xt", bufs=8, name=f"xt{kc}")
            dma(t_[:], xt_in[kc * P:(kc + 1) * P, :])
            xt.append(t_)
        wv, wq = [], []
        for kc in range(8):
            tv = sb.tile([P, INNER], BF16, tag="wv", bufs=8, name=f"wv{kc}")
            dma(tv[:], wqkv_in[kc * P:(kc + 1) * P, 2 * INNER:3 * INNER])
            wv.append(tv)
        for kc in range(8):
            tq_ = sb.tile([P, INNER], BF16, tag="wq", bufs=8, name=f"wq{kc}")
            dma(tq_[:], wqkv_in[kc * P:(kc + 1) * P, 0:INNER])
            wq.append(tq_)
            tq = sb.tile([P, mq], BF16, tag="xqt", bufs=8, name=f"xqt{kc}")
            dma(tq[:], xqt_in[kc * P:(kc + 1) * P, :])
            xqt.append(tq)

        # ---- K^T for the whole batch: kf[t] [128 featdims, Kk keys] bf16
        kf = []
        for t in range(8):
            kft = sb.tile([P, Kk], BF16, tag="kf", bufs=8, name=f"kf{t}")
            for ch in range(nkch):
                w_ = min(KCH, Kk - ch * KCH)
                kps = ps.tile([P, KCH], F32, tag="mm", name=f"kps{t}_{ch}")
                for kc in range(8):
                    nc.tensor.matmul(kps[:, 0:w_], wk[kc][:, t * P:(t + 1) * P],
                                     xt[kc][:, ch * KCH: ch * KCH + w_],
                                     start=(kc == 0), stop=(kc == 7))
                nc.vector.tensor_copy(kft[:, ch * KCH: ch * KCH + w_], kps[:, 0:w_])
            kf.append(kft)

        # ---- V~ for the whole batch: vt[kt] [128 keys, 16*(64+1)] bf16 with a
        # ones column per head (softmax denominator rides row 64 of AV psum).
        vt = []
        for kt in range(nkt):
            t_ = sb.tile([P, HEADS * (DH + 1)], BF16, tag="vt", bufs=nkt, name=f"vt{kt}")
            nc.gpsimd.memset(
                t_[:].rearrange("p (h c) -> p h c", c=DH + 1)[:, :, DH:DH + 1], 1.0)
            for nf in range(2):
                vps = ps.tile([P, 512], F32, tag="mm", name=f"vps{kt}_{nf}")
                for kc in range(8):
                    nc.tensor.matmul(vps[:], xt[kc][:, kt * P:(kt + 1) * P],
                                     wv[kc][:, nf * 512:(nf + 1) * 512],
                                     start=(kc == 0), stop=(kc == 7))
                nc.vector.tensor_copy(
                    t_[:].rearrange("p (h c) -> p h c", c=DH + 1)[:, nf * 8:(nf + 1) * 8, 0:DH],
                    vps[:].rearrange("p (h c) -> p h c", c=DH))
            vt.append(t_)

        # ---- Q^T (own slice), zero-padded per head: qtz[h] has head h's 64
        # dims in their packed partition rows, zeros in the other 64, so S^T
        # contracts over the full 128 rows sharing one K^T lhsT per head pair.
        qtz = [None] * HEADS
        for t in range(8):
            qps = ps.tile([P, mq], F32, tag="mm", name=f"qps{t}")
            for kc in range(8):
                nc.tensor.matmul(qps[:], wq[kc][:, t * P:(t + 1) * P], xqt[kc][:],
                                 start=(kc == 0), stop=(kc == 7))
            a = sb.tile([P, mq], BF16, tag="qtz", bufs=HEADS, name=f"qtz{2 * t}")
            nc.vector.memset(a[64:128, :], 0.0)
            nc.vector.tensor_copy(a[0:64, :], qps[0:64, :])
            qtz[2 * t] = a
            b = sb.tile([P, mq], BF16, tag="qtz", bufs=HEADS, name=f"qtz{2 * t + 1}")
            nc.vector.memset(b[0:64, :], 0.0)
            nc.vector.tensor_copy(b[64:128, :], qps[64:128, :])
            qtz[2 * t + 1] = b

        # ---- attention, one head-pair at a time.
        # units = (key-tile, head01); 3 units per PSUM staging tile (3 banks).
        CH = 3
        aoT = []
        for hp in range(8):
            avp_a = ps.tile([DH + 1, mq], F32, tag="av", bufs=2, name=f"av{hp}a")
            avp_b = ps.tile([DH + 1, mq], F32, tag="av", bufs=2, name=f"av{hp}b")
            avp = [avp_a, avp_b]
            units = [(kt, h) for kt in range(nkt) for h in (0, 1)]
            pending = None  # (grp, pt_) whose AV matmuls are deferred one group
            groups = [units[g0:g0 + CH] for g0 in range(0, len(units), CH)]

            def emit_av(grp, pt_):
                for j, (kt, h) in enumerate(grp):
                    nc.tensor.matmul(
                        avp[h][:],
                        vt[kt][:, (2 * hp + h) * (DH + 1):(2 * hp + h + 1) * (DH + 1)],
                        pt_[:, j * mq:(j + 1) * mq],
                        start=(kt == 0), stop=(kt == nkt - 1),
                        skip_group_check=True)

            for gi, grp in enumerate(groups):
                sps = ps.tile([P, CH * 512], F32, tag="mm", name=f"sps{hp}_{gi}")
                for j, (kt, h) in enumerate(grp):
                    nc.tensor.matmul(sps[:, j * 512: j * 512 + mq],
                                     kf[hp][:, kt * P:(kt + 1) * P],
                                     qtz[2 * hp + h][:],
                                     start=True, stop=True)
                pt_ = sb.tile([P, CH * mq], BF16, tag="pt", bufs=3, name=f"pt{hp}_{gi}")
                nc.scalar.activation(
                    pt_[:].rearrange("p (u c) -> p u c", c=mq)[:, 0:len(grp), :],
                    sps[:].rearrange("p (u c) -> p u c", c=512)[:, 0:len(grp), 0:mq],
                    mybir.ActivationFunctionType.Exp, scale=SCALE)
                if pending is not None:
                    emit_av(*pending)
                pending = (grp, pt_)
            emit_av(*pending)

            # softmax denominators ride in row 64; subtract the pad count,
            # reciprocal, broadcast across the 64 head dims, scale, pack.
            ao = sb.tile([P, mq], BF16, tag="aoT", bufs=8, name=f"aoT{hp}")
            for h, av_ in enumerate((avp_a, avp_b)):
                den = sb.tile([1, mq], F32, tag="den", bufs=4, name=f"den{hp}_{h}")
                nc.vector.tensor_scalar(den[:], av_[DH:DH + 1, :], npad_sb[0:1, 0:1],
                                        None, op0=mybir.AluOpType.subtract)
                rec = sb.tile([1, mq], F32, tag="rec", bufs=4, name=f"rec{hp}_{h}")
                nc.vector.reciprocal_approx_fast(rec[:], den[:])
                fac = sb.tile([DH, mq], F32, tag="fac", bufs=4, name=f"fac{hp}_{h}")
                nc.gpsimd.partition_broadcast(fac[:], rec[:])
                if h == 0:
                    nc.vector.tensor_tensor(ao[0:DH, :], av_[0:DH, :], fac[:],
                                            op=mybir.AluOpType.mult)
                else:
                    tmpb = sb.tile([DH, mq], BF16, tag="tmpb", bufs=2,
                                   name=f"tmpb{hp}")
                    nc.vector.tensor_tensor(tmpb[:], av_[0:DH, :], fac[:],
                                            op=mybir.AluOpType.mult)
                    # partition shift 0:64 -> 64:128 needs a DMA, not DVE
                    nc.sync.dma_start(ao[DH:P, :], tmpb[:])
            aoT.append(ao)

        # ---- out projection.
        wout_sb = []
        for t in range(8):
            tw = sb.tile([P, D], BF16, tag="wout", bufs=8, name=f"wo{t}")
            dma(tw[:], wout_in[t * P:(t + 1) * P, :])
            wout_sb.append(tw)
        for mt in range(TT):
            pm = tspan(mt)
            osb = sb.tile([P, D], F32, tag="osb", bufs=2, name=f"osb{mt}")
            for nf in range(2):
                op_ps = ps.tile([P, 512], F32, tag="mm", name=f"op{mt}_{nf}")
                for t in range(8):
                    nc.tensor.matmul(op_ps[0:pm, :], aoT[t][:, mt * P: mt * P + pm],
                                     wout_sb[t][:, nf * 512:(nf + 1) * 512],
                                     start=(t == 0), stop=(t == 7))
                nc.vector.tensor_copy(osb[0:pm, nf * 512:(nf + 1) * 512],
                                      op_ps[0:pm, :])
            nc.sync.dma_start(out_ext[mt * P: mt * P + pm, :], osb[0:pm, :])

    nc.compile()
    return nc


_GRAPH_CACHE: dict = {}


def _get_graph(mq: int):
    if mq not in _GRAPH_CACHE:
        _GRAPH_CACHE[mq] = _build(mq)
    return _GRAPH_CACHE[mq]


def kernel(x, mask, W_qkv, W_out):
    x = np.asarray(x, dtype=np.float32)
    mask = np.asarray(mask, dtype=np.float32)
    W_qkv = np.asarray(W_qkv, dtype=np.float32)
    W_out = np.asarray(W_out, dtype=np.float32)
    b, n, d = x.shape
    assert (b, d) == (2, D) and W_qkv.shape == (D, 3 * INNER)

    idx = [np.nonzero(mask[i] > 0.5)[0] for i in range(b)]
    m = [len(ix) for ix in idx]
    mq = max(32, math.ceil(max(m) / RPB / 32) * 32)
    Kk = RPB * mq

    nc = _get_graph(mq)

    bf16 = ml_dtypes.bfloat16
    xg = np.zeros((b, Kk, d), dtype=np.float32)
    for i in range(b):
        xg[i, :m[i]] = x[i][idx[i]]
    xgT = np.ascontiguousarray(xg.astype(bf16).transpose(0, 2, 1))  # [b, D, Kk]
    wqkv_bf = W_qkv.astype(bf16)
    wout_bf = W_out.astype(bf16)

    in_maps = []
    for core in range(N_CORES):
        bi, r = divmod(core, RPB)
        in_maps.append({
            "xt": xgT[bi],
            "xqt": np.ascontiguousarray(xgT[bi][:, r * mq:(r + 1) * mq]),
            "wqkv": wqkv_bf,
            "wout": wout_bf,
            "npad": np.array([[Kk - m[bi]]], dtype=np.float32),
        })

    res = run_bass_kernel_spmd(nc, in_maps, core_ids=list(range(N_CORES)))

    out = np.zeros((b, n, d), dtype=np.float32)
    for bi in range(b):
        cat = np.concatenate(
            [res.results[bi * RPB + r]["out"] for r in range(RPB)], axis=0)
        out[bi][idx[bi]] = cat[:m[bi]]
    return out


# revision 13
# speedup vs baseline: 1.7904x; 1.5293x over previous
"""Distributed Bass kernel for masked multi-head self-attention on 8 TRN2 NeuronCores.

Problem: x[2,2048,1024] -> qkv -> 16-head attention with outer-product mask
(keep[i,j] = mask[i]*mask[j]) -> out proj.  Masked queries produce exactly 0
rows and masked keys are fully excluded, so only the ~m unmasked tokens per
batch participate.  Host-side we compact tokens per batch, pad to a multiple
of 128 key slots (Kk), and split the batch dimension across the two 4-core
groups; within a group each core owns mq = Kk/4 query tokens.

Collectives on this part have a ~60-120us floor, far more than the ~25us of
extra TensorE time it takes to just recompute K and V for the whole batch on
every core of the group - so each core computes full-batch K^T and V locally
(x^T arrives host-pretransposed; no collective, no bounce buffers).

Padded token slots have x=0, so their keys score exp(0)=1 against every
query; the softmax denominator is corrected by subtracting the pad count
(a runtime input, keeping the device graph identical across cores).
Compute dtype is bf16 (f32 PSUM accumulation); softmax runs without
max-subtraction (scores are O(5), exp is safe in f32).
"""

import math
from contextlib import ExitStack

import numpy as np
import ml_dtypes

import concourse.bass as bass
import concourse.mybir as mybir
import concourse.tile as tile
from concourse import bacc
from concourse.bass_utils import run_bass_kernel_spmd
P = 128
HEADS = 16
DH = 64
D = 1024          # model dim
INNER = 1024      # heads * dh
SCALE = DH ** -0.5
N_CORES = 8
RPB = 4           # ranks (cores) per batch
BF16 = mybir.dt.bfloat16
F32 = mybir.dt.float32


def _build(mq: int, dbg: bool = False):
    """Build the per-core SPMD graph for mq queries/core (mq % 32 == 0)."""
    Kk = RPB * mq               # key slots per batch, multiple of 128
    nkt = Kk // P               # 128-row key tiles
    TT = math.ceil(mq / P)      # query-token tiles per core
    KCH = 384                   # K^T free-dim chunk (psum-bank friendly)
    nkch = math.ceil(Kk / KCH)

    def tspan(tt):
        return min(P, mq - tt * P)

    nc = bacc.Bacc(None, target_bir_lowering=False, num_devices=N_CORES)

    xt_in = nc.declare_dram_parameter("xt", [D, Kk], BF16, isOutput=False)
    xqt_in = nc.declare_dram_parameter("xqt", [D, mq], BF16, isOutput=False)
    wqkv_in = nc.declare_dram_parameter("wqkv", [D, 3 * INNER], BF16, isOutput=False)
    wout_in = nc.declare_dram_parameter("wout", [INNER, D], BF16, isOutput=False)
    npad_in = nc.declare_dram_parameter("npad", [1, 1], F32, isOutput=False)
    out_ext = nc.declare_dram_parameter("out", [mq, D], F32, isOutput=True)

    with tile.TileContext(nc) as tc, ExitStack() as ctx:
        sb = ctx.enter_context(tc.tile_pool(name="sb", bufs=1))
        ps = ctx.enter_context(tc.tile_pool(name="ps", bufs=1, space="PSUM"))

        npad_sb = sb.tile([1, 1], F32, tag="npad", bufs=1, name="npad_sb")
        nc.sync.dma_start(npad_sb[:], npad_in[:])

        # ---- inputs: x^T (full batch + own query slice), weights.
        # Round-robin the DMA issues over four sequencers (issue costs ~600ns
        # serially per sequencer); wk+xt first, they gate the first matmul.
        seqs = [nc.sync, nc.scalar, nc.gpsimd]
        _n = [0]

        def dma(dst, src):
            seqs[_n[0] % len(seqs)].dma_start(dst, src)
            _n[0] += 1

        xt, xqt, wk = [], [], []
        for kc in range(8):
            tk = sb.tile([P, INNER], BF16, tag="wk", bufs=8, name=f"wk{kc}")
            dma(tk[:], wqkv_in[kc * P:(kc + 1) * P, INNER:2 * INNER])
            wk.append(tk)
            t_ = sb.tile([P, Kk], BF16, tag="xt", bufs=8, name=f"xt{kc}")
            dma(t_[:], xt_in[kc * P:(kc + 1) * P, :])
            xt.append(t_)
        wv, wq = [], []
        for kc in range(8):
            tv = sb.tile([P, INNER], BF16, tag="wv", bufs=8, name=f"wv{kc}")
            dma(tv[:], wqkv_in[kc * P:(kc + 1) * P, 2 * INNER:3 * INNER])
            wv.append(tv)
        for kc in range(8):
            tq_ = sb.tile([P, INNER], BF16, tag="wq", bufs=8, name=f"wq{kc}")
            dma(tq_[:], wqkv_in[kc * P:(kc + 1) * P, 0:INNER])
            wq.append(tq_)
            tq = sb.tile([P, mq], BF16, tag="xqt", bufs=8, name=f"xqt{kc}")
            dma(tq[:], xqt_in[kc * P:(kc + 1) * P, :])
            xqt.append(tq)

        # ---- K^T for the whole batch: kf[t] [128 featdims, Kk keys] bf16
        kf = []
        for t in range(8):
            kft = sb.tile([P, Kk], BF16, tag="kf", bufs=8, name=f"kf{t}")
            for ch in range(nkch):
                w_ = min(KCH, Kk - ch * KCH)
                kps = ps.tile([P, KCH], F32, tag="ss", bufs=3, name=f"kps{t}_{ch}")
                for kc in range(8):
                    nc.tensor.matmul(kps[:, 0:w_], wk[kc][:, t * P:(t + 1) * P],
                                     xt[kc][:, ch * KCH: ch * KCH + w_],
                                     start=(kc == 0), stop=(kc == 7))
                nc.vector.tensor_copy(kft[:, ch * KCH: ch * KCH + w_], kps[:, 0:w_])
            kf.append(kft)

        # ---- V~ for the whole batch: vt[kt] [128 keys, 16*(64+1)] bf16 with a
        # ones column per head (softmax denominator rides row 64 of AV psum).
        vt = []
        for kt in range(nkt):
            t_ = sb.tile([P, HEADS * (DH + 1)], BF16, tag="vt", bufs=nkt, name=f"vt{kt}")
            nc.gpsimd.memset(
                t_[:].rearrange("p (h c) -> p h c", c=DH + 1)[:, :, DH:DH + 1], 1.0)
            for nf in range(2):
                vps = ps.tile([P, 512], F32, tag="ss", bufs=3, name=f"vps{kt}_{nf}")
                for kc in range(8):
                    nc.tensor.matmul(vps[:], xt[kc][:, kt * P:(kt + 1) * P],
                                     wv[kc][:, nf * 512:(nf + 1) * 512],
                                     start=(kc == 0), stop=(kc == 7))
                nc.vector.tensor_copy(
                    t_[:].rearrange("p (h c) -> p h c", c=DH + 1)[:, nf * 8:(nf + 1) * 8, 0:DH],
                    vps[:].rearrange("p (h c) -> p h c", c=DH))
            vt.append(t_)

        # ---- Q^T (own slice), zero-padded per head: qtz[h] has head h's 64
        # dims in their packed partition rows, zeros in the other 64, so S^T
        # contracts over the full 128 rows sharing one K^T lhsT per head pair.
        qtz = [None] * HEADS
        for t in range(8):
            qps = ps.tile([P, mq], F32, tag="ss", bufs=3, name=f"qps{t}")
            for kc in range(8):
                nc.tensor.matmul(qps[:], wq[kc][:, t * P:(t + 1) * P], xqt[kc][:],
                                 start=(kc == 0), stop=(kc == 7))
            a = sb.tile([P, mq], BF16, tag="qtz", bufs=HEADS, name=f"qtz{2 * t}")
            nc.vector.memset(a[64:128, :], 0.0)
            nc.vector.tensor_copy(a[0:64, :], qps[0:64, :])
            qtz[2 * t] = a
            b = sb.tile([P, mq], BF16, tag="qtz", bufs=HEADS, name=f"qtz{2 * t + 1}")
            nc.vector.memset(b[0:64, :], 0.0)
            nc.vector.tensor_copy(b[64:128, :], qps[64:128, :])
            qtz[2 * t + 1] = b

        # ---- attention, one head-pair at a time.
        # units = (key-tile, head01); 2 units per PSUM staging tile (2 banks),
        # 3 staging slots so S^T stays 2 groups ahead of the exp reads.
        CH = 2
        aoT = []
        for hp in range(8):
            avp_a = ps.tile([DH + 1, mq], F32, tag="av", bufs=2, name=f"av{hp}a")
            avp_b = ps.tile([DH + 1, mq], F32, tag="av", bufs=2, name=f"av{hp}b")
            avp = [avp_a, avp_b]
            units = [(kt, h) for kt in range(nkt) for h in (0, 1)]
            pending = None  # (grp, pt_) whose AV matmuls are deferred one group
            groups = [units[g0:g0 + CH] for g0 in range(0, len(units), CH)]

            def emit_av(grp, pt_):
                for j, (kt, h) in enumerate(grp):
                    nc.tensor.matmul(
                        avp[h][:],
                        vt[kt][:, (2 * hp + h) * (DH + 1):(2 * hp + h + 1) * (DH + 1)],
                        pt_[:, j * mq:(j + 1) * mq],
                        start=(kt == 0), stop=(kt == nkt - 1),
                        skip_group_check=True)

            for gi, grp in enumerate(groups):
                sps = ps.tile([P, CH * 512], F32, tag="ss", bufs=3, name=f"sps{hp}_{gi}")
                for j, (kt, h) in enumerate(grp):
                    nc.tensor.matmul(sps[:, j * 512: j * 512 + mq],
                                     kf[hp][:, kt * P:(kt + 1) * P],
                                     qtz[2 * hp + h][:],
                                     start=True, stop=True)
                pt_ = sb.tile([P, CH * mq], BF16, tag="pt", bufs=4, name=f"pt{hp}_{gi}")
                nc.scalar.activation(
                    pt_[:].rearrange("p (u c) -> p u c", c=mq)[:, 0:len(grp), :],
                    sps[:].rearrange("p (u c) -> p u c", c=512)[:, 0:len(grp), 0:mq],
                    mybir.ActivationFunctionType.Exp, scale=SCALE)
                if pending is not None:
                    emit_av(*pending)
                pending = (grp, pt_)
            emit_av(*pending)

            # softmax denominators ride in row 64; subtract the pad count,
            # reciprocal, broadcast across the 64 head dims, scale, pack.
            ao = sb.tile([P, mq], BF16, tag="aoT", bufs=8, name=f"aoT{hp}")
            for h, av_ in enumerate((avp_a, avp_b)):
                den = sb.tile([1, mq], F32, tag="den", bufs=4, name=f"den{hp}_{h}")
                nc.vector.tensor_scalar(den[:], av_[DH:DH + 1, :], npad_sb[0:1, 0:1],
                                        None, op0=mybir.AluOpType.subtract)
                rec = sb.tile([1, mq], F32, tag="rec", bufs=4, name=f"rec{hp}_{h}")
                nc.vector.reciprocal_approx_fast(rec[:], den[:])
                fac = sb.tile([DH, mq], F32, tag="fac", bufs=4, name=f"fac{hp}_{h}")
                nc.gpsimd.partition_broadcast(fac[:], rec[:])
                if h == 0:
                    nc.vector.tensor_tensor(ao[0:DH, :], av_[0:DH, :], fac[:],
                                            op=mybir.AluOpType.mult)
                else:
                    tmpb = sb.tile([DH, mq], BF16, tag="tmpb", bufs=2,
                                   name=f"tmpb{hp}")
                    nc.vector.tensor_tensor(tmpb[:], av_[0:DH, :], fac[:],
                                            op=mybir.AluOpType.mult)
                    # partition shift 0:64 -> 64:128 needs a DMA, not DVE
                    nc.sync.dma_start(ao[DH:P, :], tmpb[:])
            aoT.append(ao)

        # ---- out projection.
        wout_sb = []
        for t in range(8):
            tw = sb.tile([P, D], BF16, tag="wout", bufs=8, name=f"wo{t}")
            dma(tw[:], wout_in[t * P:(t + 1) * P, :])
            wout_sb.append(tw)
        for mt in range(TT):
            pm = tspan(mt)
            osb = sb.tile([P, D], F32, tag="osb", bufs=2, name=f"osb{mt}")
            for nf in range(2):
                op_ps = ps.tile([P, 512], F32, tag="ss", bufs=3, name=f"op{mt}_{nf}")
                for t in range(8):
                    nc.tensor.matmul(op_ps[0:pm, :], aoT[t][:, mt * P: mt * P + pm],
                                     wout_sb[t][:, nf * 512:(nf + 1) * 512],
                                     start=(t == 0), stop=(t == 7))
                nc.vector.tensor_copy(osb[0:pm, nf * 512:(nf + 1) * 512],
                                      op_ps[0:pm, :])
            nc.sync.dma_start(out_ext[mt * P: mt * P + pm, :], osb[0:pm, :])

    nc.compile()
    return nc


_GRAPH_CACHE: dict = {}


def _get_graph(mq: int):
    if mq not in _GRAPH_CACHE:
        _GRAPH_CACHE[mq] = _build(mq)
    return _GRAPH_CACHE[mq]


def kernel(x, mask, W_qkv, W_out):
    x = np.asarray(x, dtype=np.float32)
    mask = np.asarray(mask, dtype=np.float32)
    W_qkv = np.asarray(W_qkv, dtype=np.float32)
    W_out = np.asarray(W_out, dtype=np.float32)
    b, n, d = x.shape
    assert (b, d) == (2, D) and W_qkv.shape == (D, 3 * INNER)

    idx = [np.nonzero(mask[i] > 0.5)[0] for i in range(b)]
    m = [len(ix) for ix in idx]
    mq = max(32, math.ceil(max(m) / RPB / 32) * 32)
    Kk = RPB * mq

    nc = _get_graph(mq)

    bf16 = ml_dtypes.bfloat16
    xg = np.zeros((b, Kk, d), dtype=np.float32)
    for i in range(b):
        xg[i, :m[i]] = x[i][idx[i]]
    xgT = np.ascontiguousarray(xg.astype(bf16).transpose(0, 2, 1))  # [b, D, Kk]
    wqkv_bf = W_qkv.astype(bf16)
    wout_bf = W_out.astype(bf16)

    in_maps = []
    for core in range(N_CORES):
        bi, r = divmod(core, RPB)
        in_maps.append({
            "xt": xgT[bi],
            "xqt": np.ascontiguousarray(xgT[bi][:, r * mq:(r + 1) * mq]),
            "wqkv": wqkv_bf,
            "wout": wout_bf,
            "npad": np.array([[Kk - m[bi]]], dtype=np.float32),
        })

    res = run_bass_kernel_spmd(nc, in_maps, core_ids=list(range(N_CORES)))

    out = np.zeros((b, n, d), dtype=np.float32)
    for bi in range(b):
        cat = np.concatenate(
            [res.results[bi * RPB + r]["out"] for r in range(RPB)], axis=0)
        out[bi][idx[bi]] = cat[:m[bi]]
    return out


# revision 14
# speedup vs baseline: 1.8710x; 1.0450x over previous
"""Distributed Bass kernel for masked multi-head self-attention on 8 TRN2 NeuronCores.

Problem: x[2,2048,1024] -> qkv -> 16-head attention with outer-product mask
(keep[i,j] = mask[i]*mask[j]) -> out proj.  Masked queries produce exactly 0
rows and masked keys are fully excluded, so only the ~m unmasked tokens per
batch participate.  Host-side we compact tokens per batch, pad to a multiple
of 128 key slots (Kk), and split the batch dimension across the two 4-core
groups; within a group each core owns mq = Kk/4 query tokens.

Collectives on this part have a ~60-120us floor, far more than the ~25us of
extra TensorE time it takes to just recompute K and V for the whole batch on
every core of the group - so each core computes full-batch K^T and V locally
(x^T arrives host-pretransposed; no collective, no bounce buffers).

Padded token slots have x=0, so their keys score exp(0)=1 against every
query; the softmax denominator is corrected by subtracting the pad count
(a runtime input, keeping the device graph identical across cores).
Compute dtype is bf16 (f32 PSUM accumulation); softmax runs without
max-subtraction (scores are O(5), exp is safe in f32).
"""

import math
from contextlib import ExitStack

import numpy as np
import ml_dtypes

import concourse.bass as bass
import concourse.mybir as mybir
import concourse.tile as tile
from concourse import bacc
from concourse.bass_utils import run_bass_kernel_spmd
P = 128
HEADS = 16
DH = 64
D = 1024          # model dim
INNER = 1024      # heads * dh
SCALE = DH ** -0.5
N_CORES = 8
RPB = 4           # ranks (cores) per batch
BF16 = mybir.dt.bfloat16
F32 = mybir.dt.float32


def _build(mq: int, dbg: bool = False):
    """Build the per-core SPMD graph for mq queries/core (mq % 32 == 0)."""
    Kk = RPB * mq               # key slots per batch, multiple of 128
    nkt = Kk // P               # 128-row key tiles
    TT = math.ceil(mq / P)      # query-token tiles per core
    KCH = 384                   # K^T free-dim chunk (psum-bank friendly)
    nkch = math.ceil(Kk / KCH)

    def tspan(tt):
        return min(P, mq - tt * P)

    nc = bacc.Bacc(None, target_bir_lowering=False, num_devices=N_CORES)

    xt_in = nc.declare_dram_parameter("xt", [D, Kk], BF16, isOutput=False)
    xqt_in = nc.declare_dram_parameter("xqt", [D, mq], BF16, isOutput=False)
    wqkv_in = nc.declare_dram_parameter("wqkv", [D, 3 * INNER], BF16, isOutput=False)
    wout_in = nc.declare_dram_parameter("wout", [INNER, D], BF16, isOutput=False)
    npad_in = nc.declare_dram_parameter("npad", [1, 1], F32, isOutput=False)
    out_ext = nc.declare_dram_parameter("out", [mq, D], F32, isOutput=True)

    with tile.TileContext(nc) as tc, ExitStack() as ctx:
        sb = ctx.enter_context(tc.tile_pool(name="sb", bufs=1))
        ps = ctx.enter_context(tc.tile_pool(name="ps", bufs=1, space="PSUM"))

        npad_sb = sb.tile([1, 1], F32, tag="npad", bufs=1, name="npad_sb")
        nc.sync.dma_start(npad_sb[:], npad_in[:])

        # ---- inputs: x^T (full batch + own query slice), weights.
        # Round-robin the DMA issues over four sequencers (issue costs ~600ns
        # serially per sequencer); wk+xt first, they gate the first matmul.
        seqs = [nc.sync, nc.scalar, nc.gpsimd]
        _n = [0]

        def dma(dst, src):
            seqs[_n[0] % len(seqs)].dma_start(dst, src)
            _n[0] += 1

        xt, xqt, wk, wv, wq = [], [], [], [], []
        for kc in range(8):
            tq_ = sb.tile([P, INNER], BF16, tag="wq", bufs=8, name=f"wq{kc}")
            dma(tq_[:], wqkv_in[kc * P:(kc + 1) * P, 0:INNER])
            wq.append(tq_)
            tq = sb.tile([P, mq], BF16, tag="xqt", bufs=8, name=f"xqt{kc}")
            dma(tq[:], xqt_in[kc * P:(kc + 1) * P, :])
            xqt.append(tq)
        for kc in range(8):
            tk = sb.tile([P, INNER], BF16, tag="wk", bufs=8, name=f"wk{kc}")
            dma(tk[:], wqkv_in[kc * P:(kc + 1) * P, INNER:2 * INNER])
            wk.append(tk)
            t_ = sb.tile([P, Kk], BF16, tag="xt", bufs=8, name=f"xt{kc}")
            dma(t_[:], xt_in[kc * P:(kc + 1) * P, :])
            xt.append(t_)
        for kc in range(8):
            tv = sb.tile([P, INNER], BF16, tag="wv", bufs=8, name=f"wv{kc}")
            dma(tv[:], wqkv_in[kc * P:(kc + 1) * P, 2 * INNER:3 * INNER])
            wv.append(tv)

        # ---- Q^T (own slice), zero-padded per head: qtz[h] has head h's 64
        # dims in their packed partition rows, zeros in the other 64, so S^T
        # contracts over the full 128 rows sharing one K^T lhsT per head pair.
        qtz = [None] * HEADS
        for t in range(8):
            qps = ps.tile([P, mq], F32, tag="ss", bufs=3, name=f"qps{t}")
            for kc in range(8):
                nc.tensor.matmul(qps[:], wq[kc][:, t * P:(t + 1) * P], xqt[kc][:],
                                 start=(kc == 0), stop=(kc == 7))
            a = sb.tile([P, mq], BF16, tag="qtz", bufs=HEADS, name=f"qtz{2 * t}")
            nc.vector.memset(a[64:128, :], 0.0)
            nc.vector.tensor_copy(a[0:64, :], qps[0:64, :])
            qtz[2 * t] = a
            b = sb.tile([P, mq], BF16, tag="qtz", bufs=HEADS, name=f"qtz{2 * t + 1}")
            nc.vector.memset(b[0:64, :], 0.0)
            nc.vector.tensor_copy(b[64:128, :], qps[64:128, :])
            qtz[2 * t + 1] = b

        # ---- K^T for the whole batch: kf[t] [128 featdims, Kk keys] bf16
        kf = []
        for t in range(8):
            kft = sb.tile([P, Kk], BF16, tag="kf", bufs=8, name=f"kf{t}")
            for ch in range(nkch):
                w_ = min(KCH, Kk - ch * KCH)
                kps = ps.tile([P, KCH], F32, tag="ss", bufs=3, name=f"kps{t}_{ch}")
                for kc in range(8):
                    nc.tensor.matmul(kps[:, 0:w_], wk[kc][:, t * P:(t + 1) * P],
                                     xt[kc][:, ch * KCH: ch * KCH + w_],
                                     start=(kc == 0), stop=(kc == 7))
                nc.vector.tensor_copy(kft[:, ch * KCH: ch * KCH + w_], kps[:, 0:w_])
            kf.append(kft)

        # ---- V~ for the whole batch: vt[kt] [128 keys, 16*(64+1)] bf16 with a
        # ones column per head (softmax denominator rides row 64 of AV psum).
        vt = []
        for kt in range(nkt):
            t_ = sb.tile([P, HEADS * (DH + 1)], BF16, tag="vt", bufs=nkt, name=f"vt{kt}")
            nc.gpsimd.memset(
                t_[:].rearrange("p (h c) -> p h c", c=DH + 1)[:, :, DH:DH + 1], 1.0)
            for nf in range(2):
                vps = ps.tile([P, 512], F32, tag="ss", bufs=3, name=f"vps{kt}_{nf}")
                for kc in range(8):
                    nc.tensor.matmul(vps[:], xt[kc][:, kt * P:(kt + 1) * P],
                                     wv[kc][:, nf * 512:(nf + 1) * 512],
                                     start=(kc == 0), stop=(kc == 7))
                nc.vector.tensor_copy(
                    t_[:].rearrange("p (h c) -> p h c", c=DH + 1)[:, nf * 8:(nf + 1) * 8, 0:DH],
                    vps[:].rearrange("p (h c) -> p h c", c=DH))
            vt.append(t_)

        # ---- attention, one head-pair at a time.
        # units = (key-tile, head01); 2 units per PSUM staging tile (2 banks),
        # 3 staging slots so S^T stays 2 groups ahead of the exp reads.
        CH = 2
        aoT = []
        for hp in range(8):
            avp_a = ps.tile([DH + 1, mq], F32, tag="av", bufs=2, name=f"av{hp}a")
            avp_b = ps.tile([DH + 1, mq], F32, tag="av", bufs=2, name=f"av{hp}b")
            avp = [avp_a, avp_b]
            units = [(kt, h) for kt in range(nkt) for h in (0, 1)]
            pending = None  # (grp, pt_) whose AV matmuls are deferred one group
            groups = [units[g0:g0 + CH] for g0 in range(0, len(units), CH)]

            def emit_av(grp, pt_):
                for j, (kt, h) in enumerate(grp):
                    nc.tensor.matmul(
                        avp[h][:],
                        vt[kt][:, (2 * hp + h) * (DH + 1):(2 * hp + h + 1) * (DH + 1)],
                        pt_[:, j * mq:(j + 1) * mq],
                        start=(kt == 0), stop=(kt == nkt - 1),
                        skip_group_check=True)

            for gi, grp in enumerate(groups):
                sps = ps.tile([P, CH * 512], F32, tag="ss", bufs=3, name=f"sps{hp}_{gi}")
                for j, (kt, h) in enumerate(grp):
                    nc.tensor.matmul(sps[:, j * 512: j * 512 + mq],
                                     kf[hp][:, kt * P:(kt + 1) * P],
                                     qtz[2 * hp + h][:],
                                     start=True, stop=True)
                pt_ = sb.tile([P, CH * mq], BF16, tag="pt", bufs=4, name=f"pt{hp}_{gi}")
                nc.scalar.activation(
                    pt_[:].rearrange("p (u c) -> p u c", c=mq)[:, 0:len(grp), :],
                    sps[:].rearrange("p (u c) -> p u c", c=512)[:, 0:len(grp), 0:mq],
                    mybir.ActivationFunctionType.Exp, scale=SCALE)
                if pending is not None:
                    emit_av(*pending)
                pending = (grp, pt_)
            emit_av(*pending)

            # softmax denominators ride in row 64; subtract the pad count,
            # reciprocal, broadcast across the 64 head dims, scale, pack.
            # Copy PSUM->SBUF first so the next pair's AV psum slot frees after
            # ~450ns instead of after the whole normalize chain.
            ao = sb.tile([P, mq], BF16, tag="aoT", bufs=8, name=f"aoT{hp}")
            for h, avp_h in enumerate((avp_a, avp_b)):
                av_ = sb.tile([DH + 1, mq], F32, tag="aos", bufs=4, name=f"aos{hp}_{h}")
                nc.vector.tensor_copy(av_[:], avp_h[:])
                den = sb.tile([1, mq], F32, tag="den", bufs=4, name=f"den{hp}_{h}")
                nc.vector.tensor_scalar(den[:], av_[DH:DH + 1, :], npad_sb[0:1, 0:1],
                                        None, op0=mybir.AluOpType.subtract)
                rec = sb.tile([1, mq], F32, tag="rec", bufs=4, name=f"rec{hp}_{h}")
                nc.vector.reciprocal_approx_fast(rec[:], den[:])
                fac = sb.tile([DH, mq], F32, tag="fac", bufs=4, name=f"fac{hp}_{h}")
                nc.gpsimd.partition_broadcast(fac[:], rec[:])
                if h == 0:
                    nc.vector.tensor_tensor(ao[0:DH, :], av_[0:DH, :], fac[:],
                                            op=mybir.AluOpType.mult)
                else:
                    tmpb = sb.tile([DH, mq], BF16, tag="tmpb", bufs=2,
                                   name=f"tmpb{hp}")
                    nc.vector.tensor_tensor(tmpb[:], av_[0:DH, :], fac[:],
                                            op=mybir.AluOpType.mult)
                    # partition shift 0:64 -> 64:128 needs a DMA, not DVE
                    nc.sync.dma_start(ao[DH:P, :], tmpb[:])
            aoT.append(ao)

        # ---- out projection.
        wout_sb = []
        for t in range(8):
            tw = sb.tile([P, D], BF16, tag="wout", bufs=8, name=f"wo{t}")
            dma(tw[:], wout_in[t * P:(t + 1) * P, :])
            wout_sb.append(tw)
        for mt in range(TT):
            pm = tspan(mt)
            osb = sb.tile([P, D], F32, tag="osb", bufs=2, name=f"osb{mt}")
            for nf in range(2):
                op_ps = ps.tile([P, 512], F32, tag="ss", bufs=3, name=f"op{mt}_{nf}")
                for t in range(8):
                    nc.tensor.matmul(op_ps[0:pm, :], aoT[t][:, mt * P: mt * P + pm],
                                     wout_sb[t][:, nf * 512:(nf + 1) * 512],
                                     start=(t == 0), stop=(t == 7))
                nc.vector.tensor_copy(osb[0:pm, nf * 512:(nf + 1) * 512],
                                      op_ps[0:pm, :])
            nc.sync.dma_start(out_ext[mt * P: mt * P + pm, :], osb[0:pm, :])

    nc.compile()
    return nc


_GRAPH_CACHE: dict = {}


def _get_graph(mq: int):
    if mq not in _GRAPH_CACHE:
        _GRAPH_CACHE[mq] = _build(mq)
    return _GRAPH_CACHE[mq]


def kernel(x, mask, W_qkv, W_out):
    x = np.asarray(x, dtype=np.float32)
    mask = np.asarray(mask, dtype=np.float32)
    W_qkv = np.asarray(W_qkv, dtype=np.float32)
    W_out = np.asarray(W_out, dtype=np.float32)
    b, n, d = x.shape
    assert (b, d) == (2, D) and W_qkv.shape == (D, 3 * INNER)

    idx = [np.nonzero(mask[i] > 0.5)[0] for i in range(b)]
    m = [len(ix) for ix in idx]
    mq = max(32, math.ceil(max(m) / RPB / 32) * 32)
    Kk = RPB * mq

    nc = _get_graph(mq)

    bf16 = ml_dtypes.bfloat16
    xg = np.zeros((b, Kk, d), dtype=np.float32)
    for i in range(b):
        xg[i, :m[i]] = x[i][idx[i]]
    xgT = np.ascontiguousarray(xg.astype(bf16).transpose(0, 2, 1))  # [b, D, Kk]
    wqkv_bf = W_qkv.astype(bf16)
    wout_bf = W_out.astype(bf16)

    in_maps = []
    for core in range(N_CORES):
        bi, r = divmod(core, RPB)
        in_maps.append({
            "xt": xgT[bi],
            "xqt": np.ascontiguousarray(xgT[bi][:, r * mq:(r + 1) * mq]),
            "wqkv": wqkv_bf,
            "wout": wout_bf,
            "npad": np.array([[Kk - m[bi]]], dtype=np.float32),
        })

    res = run_bass_kernel_spmd(nc, in_maps, core_ids=list(range(N_CORES)))

    out = np.zeros((b, n, d), dtype=np.float32)
    for bi in range(b):
        cat = np.concatenate(
            [res.results[bi * RPB + r]["out"] for r in range(RPB)], axis=0)
        out[bi][idx[bi]] = cat[:m[bi]]
    return out


# revision 15
# speedup vs baseline: 1.8868x; 1.0084x over previous
"""Distributed Bass kernel for masked multi-head self-attention on 8 TRN2 NeuronCores.

Problem: x[2,2048,1024] -> qkv -> 16-head attention with outer-product mask
(keep[i,j] = mask[i]*mask[j]) -> out proj.  Masked queries produce exactly 0
rows and masked keys are fully excluded, so only the ~m unmasked tokens per
batch participate.  Host-side we compact tokens per batch, pad to a multiple
of 128 key slots (Kk), and split the batch dimension across the two 4-core
groups; within a group each core owns mq = Kk/4 query tokens.

Collectives on this part have a ~60-120us floor, far more than the ~25us of
extra TensorE time it takes to just recompute K and V for the whole batch on
every core of the group - so each core computes full-batch K^T and V locally
(x^T arrives host-pretransposed; no collective, no bounce buffers).

Padded token slots have x=0, so their keys score exp(0)=1 against every
query; the softmax denominator is corrected by subtracting the pad count
(a runtime input, keeping the device graph identical across cores).
Compute dtype is bf16 (f32 PSUM accumulation); softmax runs without
max-subtraction (scores are O(5), exp is safe in f32).
"""

import math
from contextlib import ExitStack

import numpy as np
import ml_dtypes

import concourse.bass as bass
import concourse.mybir as mybir
import concourse.tile as tile
from concourse import bacc
from concourse.bass_utils import run_bass_kernel_spmd
P = 128
HEADS = 16
DH = 64
D = 1024          # model dim
INNER = 1024      # heads * dh
SCALE = DH ** -0.5
N_CORES = 8
RPB = 4           # ranks (cores) per batch
BF16 = mybir.dt.bfloat16
F32 = mybir.dt.float32


def _build(mq: int, dbg: bool = False):
    """Build the per-core SPMD graph for mq queries/core (mq % 32 == 0)."""
    Kk = RPB * mq               # key slots per batch, multiple of 128
    nkt = Kk // P               # 128-row key tiles
    TT = math.ceil(mq / P)      # query-token tiles per core
    KCH = 384                   # K^T free-dim chunk (psum-bank friendly)
    nkch = math.ceil(Kk / KCH)

    def tspan(tt):
        return min(P, mq - tt * P)

    nc = bacc.Bacc(None, target_bir_lowering=False, num_devices=N_CORES)

    xt_in = nc.declare_dram_parameter("xt", [D, Kk], BF16, isOutput=False)
    xqt_in = nc.declare_dram_parameter("xqt", [D, mq], BF16, isOutput=False)
    wqkv_in = nc.declare_dram_parameter("wqkv", [D, 3 * INNER], BF16, isOutput=False)
    wout_in = nc.declare_dram_parameter("wout", [INNER, D], BF16, isOutput=False)
    npad_in = nc.declare_dram_parameter("npad", [1, 1], F32, isOutput=False)
    out_ext = nc.declare_dram_parameter("out", [mq, D], F32, isOutput=True)

    with tile.TileContext(nc) as tc, ExitStack() as ctx:
        sb = ctx.enter_context(tc.tile_pool(name="sb", bufs=1))
        ps = ctx.enter_context(tc.tile_pool(name="ps", bufs=1, space="PSUM"))

        npad_sb = sb.tile([1, 1], F32, tag="npad", bufs=1, name="npad_sb")
        nc.sync.dma_start(npad_sb[:], npad_in[:])

        # ---- inputs: x^T (full batch + own query slice), weights.
        # Round-robin the DMA issues over four sequencers (issue costs ~600ns
        # serially per sequencer); wk+xt first, they gate the first matmul.
        seqs = [nc.sync, nc.scalar, nc.gpsimd]
        _n = [0]

        def dma(dst, src):
            seqs[_n[0] % len(seqs)].dma_start(dst, src)
            _n[0] += 1

        xt, xqt, wk, wv, wq = [], [], [], [], []
        for kc in range(8):
            tq_ = sb.tile([P, INNER], BF16, tag="wq", bufs=8, name=f"wq{kc}")
            dma(tq_[:], wqkv_in[kc * P:(kc + 1) * P, 0:INNER])
            wq.append(tq_)
            tq = sb.tile([P, mq], BF16, tag="xqt", bufs=8, name=f"xqt{kc}")
            dma(tq[:], xqt_in[kc * P:(kc + 1) * P, :])
            xqt.append(tq)
        for kc in range(8):
            tk = sb.tile([P, INNER], BF16, tag="wk", bufs=8, name=f"wk{kc}")
            dma(tk[:], wqkv_in[kc * P:(kc + 1) * P, INNER:2 * INNER])
            wk.append(tk)
            t_ = sb.tile([P, Kk], BF16, tag="xt", bufs=8, name=f"xt{kc}")
            dma(t_[:], xt_in[kc * P:(kc + 1) * P, :])
            xt.append(t_)
        for kc in range(8):
            tv = sb.tile([P, INNER], BF16, tag="wv", bufs=8, name=f"wv{kc}")
            dma(tv[:], wqkv_in[kc * P:(kc + 1) * P, 2 * INNER:3 * INNER])
            wv.append(tv)

        wout_sb = []
        for t in range(8):
            tw = sb.tile([P, D], BF16, tag="wout", bufs=8, name=f"wo{t}")
            dma(tw[:], wout_in[t * P:(t + 1) * P, :])
            wout_sb.append(tw)

        # ---- Q^T (own slice), zero-padded per head: qtz[h] has head h's 64
        # dims in their packed partition rows, zeros in the other 64, so S^T
        # contracts over the full 128 rows sharing one K^T lhsT per head pair.
        qtz = [None] * HEADS
        for t in range(8):
            qps = ps.tile([P, mq], F32, tag="ss", bufs=3, name=f"qps{t}")
            kcs = [(t + i) % 8 for i in range(8)]
            for i, kc in enumerate(kcs):
                nc.tensor.matmul(qps[:], wq[kc][:, t * P:(t + 1) * P], xqt[kc][:],
                                 start=(i == 0), stop=(i == 7))
            a = sb.tile([P, mq], BF16, tag="qtz", bufs=HEADS, name=f"qtz{2 * t}")
            nc.vector.memset(a[64:128, :], 0.0)
            nc.vector.tensor_copy(a[0:64, :], qps[0:64, :])
            qtz[2 * t] = a
            b = sb.tile([P, mq], BF16, tag="qtz", bufs=HEADS, name=f"qtz{2 * t + 1}")
            nc.vector.memset(b[0:64, :], 0.0)
            nc.vector.tensor_copy(b[64:128, :], qps[64:128, :])
            qtz[2 * t + 1] = b

        # ---- K^T for the whole batch: kf[t] [128 featdims, Kk keys] bf16
        kf = []
        for t in range(8):
            kft = sb.tile([P, Kk], BF16, tag="kf", bufs=8, name=f"kf{t}")
            for ch in range(nkch):
                w_ = min(KCH, Kk - ch * KCH)
                kps = ps.tile([P, KCH], F32, tag="ss", bufs=3, name=f"kps{t}_{ch}")
                kcs = [(t + ch + i) % 8 for i in range(8)]
                for i, kc in enumerate(kcs):
                    nc.tensor.matmul(kps[:, 0:w_], wk[kc][:, t * P:(t + 1) * P],
                                     xt[kc][:, ch * KCH: ch * KCH + w_],
                                     start=(i == 0), stop=(i == 7))
                nc.vector.tensor_copy(kft[:, ch * KCH: ch * KCH + w_], kps[:, 0:w_])
            kf.append(kft)

        # ---- V~ for the whole batch: vt[kt] [128 keys, 16*(64+1)] bf16 with a
        # ones column per head (softmax denominator rides row 64 of AV psum).
        vt = []
        for kt in range(nkt):
            t_ = sb.tile([P, HEADS * (DH + 1)], BF16, tag="vt", bufs=nkt, name=f"vt{kt}")
            nc.gpsimd.memset(
                t_[:].rearrange("p (h c) -> p h c", c=DH + 1)[:, :, DH:DH + 1], 1.0)
            for nf in range(2):
                vps = ps.tile([P, 512], F32, tag="ss", bufs=3, name=f"vps{kt}_{nf}")
                kcs = [(kt + nf + i) % 8 for i in range(8)]
                for i, kc in enumerate(kcs):
                    nc.tensor.matmul(vps[:], xt[kc][:, kt * P:(kt + 1) * P],
                                     wv[kc][:, nf * 512:(nf + 1) * 512],
                                     start=(i == 0), stop=(i == 7))
                nc.vector.tensor_copy(
                    t_[:].rearrange("p (h c) -> p h c", c=DH + 1)[:, nf * 8:(nf + 1) * 8, 0:DH],
                    vps[:].rearrange("p (h c) -> p h c", c=DH))
            vt.append(t_)

        # ---- attention, one head-pair at a time.
        # units = (key-tile, head01); 2 units per PSUM staging tile (2 banks),
        # 3 staging slots so S^T stays 2 groups ahead of the exp reads.
        CH = 2
        aoT = []
        for hp in range(8):
            avp_a = ps.tile([DH + 1, mq], F32, tag="av", bufs=2, name=f"av{hp}a")
            avp_b = ps.tile([DH + 1, mq], F32, tag="av", bufs=2, name=f"av{hp}b")
            avp = [avp_a, avp_b]
            units = [(kt, h) for kt in range(nkt) for h in (0, 1)]
            pending = None  # (grp, pt_) whose AV matmuls are deferred one group
            groups = [units[g0:g0 + CH] for g0 in range(0, len(units), CH)]

            def emit_av(grp, pt_):
                for j, (kt, h) in enumerate(grp):
                    nc.tensor.matmul(
                        avp[h][:],
                        vt[kt][:, (2 * hp + h) * (DH + 1):(2 * hp + h + 1) * (DH + 1)],
                        pt_[:, j * mq:(j + 1) * mq],
                        start=(kt == 0), stop=(kt == nkt - 1),
                        skip_group_check=True)

            for gi, grp in enumerate(groups):
                sps = ps.tile([P, CH * 512], F32, tag="ss", bufs=3, name=f"sps{hp}_{gi}")
                for j, (kt, h) in enumerate(grp):
                    nc.tensor.matmul(sps[:, j * 512: j * 512 + mq],
                                     kf[hp][:, kt * P:(kt + 1) * P],
                                     qtz[2 * hp + h][:],
                                     start=True, stop=True)
                pt_ = sb.tile([P, CH * mq], BF16, tag="pt", bufs=4, name=f"pt{hp}_{gi}")
                nc.scalar.activation(
                    pt_[:].rearrange("p (u c) -> p u c", c=mq)[:, 0:len(grp), :],
                    sps[:].rearrange("p (u c) -> p u c", c=512)[:, 0:len(grp), 0:mq],
                    mybir.ActivationFunctionType.Exp, scale=SCALE)
                if pending is not None:
                    emit_av(*pending)
                pending = (grp, pt_)
            emit_av(*pending)

            # softmax denominators ride in row 64; subtract the pad count,
            # reciprocal, broadcast across the 64 head dims, scale, pack.
            # Copy PSUM->SBUF first so the next pair's AV psum slot frees after
            # ~450ns instead of after the whole normalize chain.
            ao = sb.tile([P, mq], BF16, tag="aoT", bufs=8, name=f"aoT{hp}")
            for h, avp_h in enumerate((avp_a, avp_b)):
                av_ = sb.tile([DH + 1, mq], F32, tag="aos", bufs=4, name=f"aos{hp}_{h}")
                nc.vector.tensor_copy(av_[:], avp_h[:])
                den = sb.tile([1, mq], F32, tag="den", bufs=4, name=f"den{hp}_{h}")
                nc.vector.tensor_scalar(den[:], av_[DH:DH + 1, :], npad_sb[0:1, 0:1],
                                        None, op0=mybir.AluOpType.subtract)
                rec = sb.tile([1, mq], F32, tag="rec", bufs=4, name=f"rec{hp}_{h}")
                nc.vector.reciprocal_approx_fast(rec[:], den[:])
                fac = sb.tile([DH, mq], F32, tag="fac", bufs=4, name=f"fac{hp}_{h}")
                nc.gpsimd.partition_broadcast(fac[:], rec[:])
                if h == 0:
                    nc.vector.tensor_tensor(ao[0:DH, :], av_[0:DH, :], fac[:],
                                            op=mybir.AluOpType.mult)
                else:
                    tmpb = sb.tile([DH, mq], BF16, tag="tmpb", bufs=2,
                                   name=f"tmpb{hp}")
                    nc.vector.tensor_tensor(tmpb[:], av_[0:DH, :], fac[:],
                                            op=mybir.AluOpType.mult)
                    # partition shift 0:64 -> 64:128 needs a DMA, not DVE
                    nc.sync.dma_start(ao[DH:P, :], tmpb[:])
            aoT.append(ao)

        # ---- out projection.
        for mt in range(TT):
            pm = tspan(mt)
            osb = sb.tile([P, D], F32, tag="osb", bufs=2, name=f"osb{mt}")
            for nf in range(2):
                op_ps = ps.tile([P, 512], F32, tag="ss", bufs=3, name=f"op{mt}_{nf}")
                for t in range(8):
                    nc.tensor.matmul(op_ps[0:pm, :], aoT[t][:, mt * P: mt * P + pm],
                                     wout_sb[t][:, nf * 512:(nf + 1) * 512],
                                     start=(t == 0), stop=(t == 7))
                nc.vector.tensor_copy(osb[0:pm, nf * 512:(nf + 1) * 512],
                                      op_ps[0:pm, :])
            nc.sync.dma_start(out_ext[mt * P: mt * P + pm, :], osb[0:pm, :])

    nc.compile()
    return nc


_GRAPH_CACHE: dict = {}


def _get_graph(mq: int):
    if mq not in _GRAPH_CACHE:
        _GRAPH_CACHE[mq] = _build(mq)
    return _GRAPH_CACHE[mq]


def kernel(x, mask, W_qkv, W_out):
    x = np.asarray(x, dtype=np.float32)
    mask = np.asarray(mask, dtype=np.float32)
    W_qkv = np.asarray(W_qkv, dtype=np.float32)
    W_out = np.asarray(W_out, dtype=np.float32)
    b, n, d = x.shape
    assert (b, d) == (2, D) and W_qkv.shape == (D, 3 * INNER)

    idx = [np.nonzero(mask[i] > 0.5)[0] for i in range(b)]
    m = [len(ix) for ix in idx]
    mq = max(32, math.ceil(max(m) / RPB / 32) * 32)
    Kk = RPB * mq

    nc = _get_graph(mq)

    bf16 = ml_dtypes.bfloat16
    xg = np.zeros((b, Kk, d), dtype=np.float32)
    for i in range(b):
        xg[i, :m[i]] = x[i][idx[i]]
    xgT = np.ascontiguousarray(xg.astype(bf16).transpose(0, 2, 1))  # [b, D, Kk]
    wqkv_bf = W_qkv.astype(bf16)
    wout_bf = W_out.astype(bf16)

    in_maps = []
    for core in range(N_CORES):
        bi, r = divmod(core, RPB)
        in_maps.append({
            "xt": xgT[bi],
            "xqt": np.ascontiguousarray(xgT[bi][:, r * mq:(r + 1) * mq]),
            "wqkv": wqkv_bf,
            "wout": wout_bf,
            "npad": np.array([[Kk - m[bi]]], dtype=np.float32),
        })

    res = run_bass_kernel_spmd(nc, in_maps, core_ids=list(range(N_CORES)))

    out = np.zeros((b, n, d), dtype=np.float32)
    for bi in range(b):
        cat = np.concatenate(
            [res.results[bi * RPB + r]["out"] for r in range(RPB)], axis=0)
        out[bi][idx[bi]] = cat[:m[bi]]
    return out
